# revision 3
# baseline (speedup 1.0000x reference)
"""Trainium2 Bass kernel for nn_GroupLocalSL2 (grouped gather + conv).

out[b,o,i,xo,yo] = sum_{c,f,kh,kw} x[b,c,idx[i,f],xo+kh,yo+kw] * W[o,c,f,kh,kw] + bias[o]

Strategy:
  - Batch B=8 sharded across 8 NeuronCores (data parallel), one b per core.
  - Host pre-gathers x per output group (idx applied host-side) into
    contiguous [G_OUT, 128|96, X, Y+1] tensors so each group needs just two
    large DMAs instead of seven small gather DMAs (dma_start dispatch is
    ~590ns serial per issuing queue).
  - Per core: contraction (c,f)=224 split into chunkA (f=0..3, K=128) and
    chunkB (f=4..6, K=96), partition p = f*32 + c.
  - kw offsets packed into matmul M-blocks: {kw0,kw1} and {kw2,kw3} give
    M=128 matmuls; kw4 runs as two concurrent M=64 col-tiled matmuls. kh
    accumulates in PSUM via row-shifted rhs windows.
  - All 30 matmuls of a row-chunk accumulate into ONE psum tile [128, R, 61]:
    {kw2,kw3} streams x cols 2:63 so it lands at the same psum columns as
    {kw0,kw1}; the kw4 pair lands even-aligned at [0:64] (x cols 4:65 via a
    zero-padded 65th column) and odd-aligned at [64:128] (x cols 3:64).
    Combine is then 2 ops: ScalarE bias-add of the even half [.., 0:60] plus
    VectorE add of the odd half [.., 1:61].
  - Per group, all 8 row-chunks' M=128 matmuls are issued first (phase 1),
    then the M=64 kw4 pairs (phase 2), with one psum bank per row-chunk: the
    PE pays its ~110ns tile-config switch penalty twice per group instead of
    twice per row-chunk.
  - Head: critical DMAs split across BOTH HWDGE dispatch queues (SyncE for
    x bands, ScalarE/ACT for weights) so the serial ~590ns dispatch chains
    run in parallel; warmup matmul count sized so the PE queue drains right
    as the group-0 data lands (~11us), instead of over-warming.
  - Tail: last group's output DMAs alternate Sync/Scalar dispatch queues and
    the final row-chunk's odd-half add runs on the idle GpSimd engine so the
    drain after the last matmul is short.
  - PE warmed up with dummy matmuls during the initial DMA fill (HAM clock
    gate holds the array at 1.2 GHz until ~3.4us of sustained activity).
  - Compute in bf16 (host casts x/W), fp32 PSUM accumulate; rel err ~2e-3.

  Falsified alternatives (measured on HW, all reverted — do not retry):
  - Per-kw-block weight tiles (wa0/wa1/..): +120us — splitting the combined
    weight tiles breaks LDWEIGHTS/FWL overlap (+42ns on EVERY matmul).
  - Host-prewindowed pitch-61 x variants for flat (crossing-free) rhs:
    +152us — the 2.8x gather traffic stalls matmuls; prefetch cannot hide it.
  - fp8 (any hi/lo split passing rel-err 2e-2 is >=1.5x bf16 MACs), Winograd
    (transforms cannot ride the PE; DVE is 100x too slow), kh-in-K packing
    (needs 5x row-shifted x copies): all slower at equal accuracy.
"""

import os
import sys

import numpy as np
import ml_dtypes

for _p in ("/opt/trn_rl_repo", "/root/.axon_site/_ro/trn_rl_repo"):
    if os.path.isdir(_p) and _p not in sys.path:
        sys.path.append(_p)

import concourse.bass as bass
import concourse.mybir as mybir
import concourse.tile as tile
from concourse import bacc
from concourse.bass_utils import run_bass_kernel_spmd

BF16 = ml_dtypes.bfloat16

B, C, G_IN = 8, 32, 33
O, G_F, KH, KW = 64, 7, 5, 5
X, Y = 64, 64
G_OUT = 15
XO, YO = X - KH + 1, Y - KW + 1  # 60, 60
RCH = 8  # output rows per chunk (8*61 = 488 <= 512 psum bank)
N_WARM = 16  # dummy matmuls to flip the HAM clock gate during DMA fill


def _build_nc(n_groups=G_OUT):
    """Build the single-core Bass program (x pre-gathered host-side)."""
    nc = bacc.Bacc("TRN2", target_bir_lowering=False, debug=False)
    dt = mybir.dt
    ga_d = nc.dram_tensor("ga", [G_OUT, 128, X, Y + 1], dt.bfloat16, kind="ExternalInput")
    gb_d = nc.dram_tensor("gb", [G_OUT, 96, X, Y + 1], dt.bfloat16, kind="ExternalInput")
    wa_d = nc.dram_tensor("wa", [128, KH, 5 * O], dt.bfloat16, kind="ExternalInput")
    wb_d = nc.dram_tensor("wb", [96, KH, 5 * O], dt.bfloat16, kind="ExternalInput")
    bias_d = nc.dram_tensor("bias", [O, 1], dt.float32, kind="ExternalInput")
    out_d = nc.dram_tensor("out", [O, G_OUT, XO, YO], dt.float32, kind="ExternalOutput")

    rchunks = [(r0, min(RCH, XO - r0)) for r0 in range(0, XO, RCH)]

    with tile.TileContext(nc) as tc:
        with (
            tc.tile_pool(name="wpool", bufs=1) as wpool,
            tc.tile_pool(name="warm", bufs=1) as warmpool,
            tc.tile_pool(name="xpool", bufs=2) as xpool,
            tc.tile_pool(name="tpool", bufs=3) as tpool,
            tc.tile_pool(name="opool", bufs=4) as opool,
            tc.tile_pool(name="psum", bufs=7, space="PSUM") as pp,
            tc.tile_pool(name="psumt", bufs=1, space="PSUM") as ppt,
        ):
            # PE warmup: the HAM clock gate holds the PE at 1.2 GHz until it
            # sees ~3.4us of sustained activity. Burn that in on garbage data
            # while the weight/x DMAs fill SBUF.
            wmt = warmpool.tile([128, 256], dt.bfloat16, tag="warm")
            nc.vector.memset(wmt[:, :], 0.0)

            # Weight DMAs ride the ScalarE HWDGE queue so their dispatch
            # chain runs in parallel with the x-band dispatches on SyncE.
            wa01 = wpool.tile([128, KH, 4 * O], dt.bfloat16, tag="wa01")
            wa4 = wpool.tile([128, KH, O], dt.bfloat16, tag="wa4")
            wb = wpool.tile([96, KH, 5 * O], dt.bfloat16, tag="wb")
            bias_sb = wpool.tile([O, 1], dt.float32, tag="bias")
            nc.scalar.dma_start(wa01[:, :, :], wa_d[:, :, 0 : 4 * O])
            nc.scalar.dma_start(wb[:, :, :], wb_d[:, :, :])
            nc.scalar.dma_start(wa4[:, :, :], wa_d[:, :, 4 * O : 5 * O])
            nc.scalar.dma_start(bias_sb[:, :], bias_d[:, :])

            for i in range(n_groups):
                # xa has a 65th zeroed column so the kw4 matmul can stream a
                # full 61-wide window (cols 4:65) for a contiguous psum write.
                xa = xpool.tile([128, X, Y + 1], dt.bfloat16, tag="xa")
                xb = xpool.tile([96, X, Y + 1], dt.bfloat16, tag="xb")
                # group 0 in two row bands: the first band unblocks the first
                # row chunks' matmuls sooner
                bands = ((0, 24), (24, X)) if i == 0 else ((0, X),)
                for lo, hi in bands:
                    nc.sync.dma_start(xa[:, lo:hi, :], ga_d[i, :, lo:hi, :])
                    nc.sync.dma_start(xb[:, lo:hi, :], gb_d[i, :, lo:hi, :])
                if i == 0:
                    # warmup matmuls run while the DMAs above land
                    wps = ppt.tile([128, 4, 61], dt.float32, tag="pt")
                    for _ in range(N_WARM):
                        nc.tensor.matmul(
                            wps[:, :, :],
                            wmt[:, 0:128],
                            wmt[:, 0:244],
                            start=True,
                            stop=True,
                        )

                # Phase 1: the M=128 {kw0,kw1}/{kw2,kw3} blocks of ALL row
                # chunks, one psum bank per chunk. Phase 2: the M=64 kw4
                # col-tiled pairs. Keeping all M=128 matmuls together avoids
                # the ~110ns PE tile-config switch penalty on every M change
                # (2 per group instead of 2 per row chunk).
                ptiles = []
                for r0, R in rchunks:
                    tail = R != RCH
                    p = (ppt if tail else pp).tile(
                        [128, R, 61], dt.float32, tag="pt" if tail else "p"
                    )
                    ptiles.append(p)
                    # {kw2,kw3} streams x cols 2:63 so its contributions land
                    # at the same psum columns as {kw0,kw1}.
                    for grp in (0, 1):
                        c0 = 2 * grp
                        for ci, (xt, wt, Kc) in enumerate(
                            ((xa, wa01, 128), (xb, wb, 96))
                        ):
                            for kh in range(KH):
                                nc.tensor.matmul(
                                    p[:, 0:R, :],
                                    wt[0:Kc, kh, grp * 128 : grp * 128 + 128],
                                    xt[0:Kc, r0 + kh : r0 + kh + R, c0 : c0 + 61],
                                    start=(grp == 0 and ci == 0 and kh == 0),
                                    stop=False,
                                )
                for ri, ((r0, R), p) in enumerate(zip(rchunks, ptiles)):
                    # kw4 col-tiled pair: even-aligned at [0:64] (x cols 4:65,
                    # zero-padded 65th col), odd-aligned at [64:128] (cols
                    # 3:64).
                    for kh in range(KH):
                        nc.tensor.matmul(
                            p[0:64, 0:R, :],
                            wa4[0:128, kh, 0:64],
                            xa[0:128, r0 + kh : r0 + kh + R, 4:65],
                            start=False,
                            stop=False,
                        )
                        nc.tensor.matmul(
                            p[64:128, 0:R, :],
                            wb[0:96, kh, 256:320],
                            xb[0:96, r0 + kh : r0 + kh + R, 3:64],
                            start=False,
                            stop=(kh == KH - 1),
                        )

                    # Combine (one PSUM operand per instruction): bias-add of
                    # the even half on ScalarE, odd half on VectorE.
                    # (GpSimd cannot read PSUM on TRN2, so no third engine.)
                    t = tpool.tile([O, RCH, 60], dt.float32, tag="t")
                    ot = opool.tile([O, RCH, 60], dt.float32, tag="out")
                    nc.scalar.add(t[:, 0:R, :], p[0:64, 0:R, 0:60], bias_sb[:, 0:1])
                    nc.vector.tensor_add(
                        ot[:, 0:R, :], t[:, 0:R, :], p[64:128, 0:R, 1:61]
                    )
                    # Output DMAs alternate HWDGE dispatch queues so the
                    # ~590ns serial dispatch chains halve, which matters for
                    # the post-final-matmul drain of the last group.
                    oeng = nc.sync if ri % 2 == 0 else nc.scalar
                    oeng.dma_start(out_d[:, i, r0 : r0 + R, :], ot[:, 0:R, :])
    nc.compile()
    return nc


def _prep_inputs(x, weight, bias, idx):
    """Host-side staging: bf16 cast, idx gather, lhsT weight layout."""
    x16 = np.asarray(x).astype(BF16)  # [B, C, G_IN, X, Y]
    # pad a zero 65th column so kw4's 61-wide window (cols 4:65) exists
    x16 = np.pad(x16, ((0, 0), (0, 0), (0, 0), (0, 0), (0, 1)))
    w = np.asarray(weight).astype(np.float32)
    # lhsT layout: partition p = f*32 + c (within chunk), free = [kh, kw*64+o]
    wt = w.transpose(2, 1, 3, 4, 0)  # [G_F, C, KH, KW, O]
    wa = np.ascontiguousarray(wt[0:4].reshape(128, KH, KW * O)).astype(BF16)
    wb = np.ascontiguousarray(wt[4:7].reshape(96, KH, KW * O)).astype(BF16)
    b2 = np.ascontiguousarray(np.asarray(bias).astype(np.float32).reshape(O, 1))
    in_maps = []
    for b in range(B):
        # gather: g[i, f*32+c] = x16[b, c, idx[i, f]]
        gx = x16[b][:, idx]  # [C, G_OUT, G_F, X, Y+1]
        gx = gx.transpose(1, 2, 0, 3, 4).reshape(G_OUT, G_F * C, X, Y + 1)
        in_maps.append(
            {
                "ga": np.ascontiguousarray(gx[:, 0:128]),
                "gb": np.ascontiguousarray(gx[:, 128:224]),
                "wa": wa,
                "wb": wb,
                "bias": b2,
            }
        )
    return in_maps


def run(x, weight, bias, idx, trace=False):
    idx = np.asarray(idx).astype(np.int64)
    assert idx.shape == (G_OUT, G_F) and idx.min() >= 0 and idx.max() < G_IN
    nc = _build_nc()
    in_maps = _prep_inputs(x, weight, bias, idx)
    res = run_bass_kernel_spmd(nc, in_maps, list(range(B)), trace=trace)
    out = np.stack([res.results[b]["out"] for b in range(B)]).astype(np.float32)
    return out, res


def kernel(x, weight, bias, idx):
    out, _ = run(x, weight, bias, idx, trace=False)
    return out


# revision 7
# speedup vs baseline: 1.0068x; 1.0068x over previous
"""Trainium2 Bass kernel for nn_GroupLocalSL2 (grouped gather + conv).

out[b,o,i,xo,yo] = sum_{c,f,kh,kw} x[b,c,idx[i,f],xo+kh,yo+kw] * W[o,c,f,kh,kw] + bias[o]

Strategy:
  - Batch B=8 sharded across 8 NeuronCores (data parallel), one b per core.
  - Host pre-gathers x per output group (idx applied host-side) into
    contiguous [G_OUT, 128|96, X, Y+1] tensors so each group needs just two
    large DMAs instead of seven small gather DMAs (dma_start dispatch is
    ~590ns serial per issuing queue).
  - Per core: contraction (c,f)=224 split into chunkA (f=0..3, K=128) and
    chunkB (f=4..6, K=96), partition p = f*32 + c.
  - kw offsets packed into matmul M-blocks: {kw0,kw1} and {kw2,kw3} give
    M=128 matmuls; kw4 runs as two concurrent M=64 col-tiled matmuls. kh
    accumulates in PSUM via row-shifted rhs windows.
  - All 30 matmuls of a row-chunk accumulate into ONE psum tile [128, R, 61]:
    {kw2,kw3} streams x cols 2:63 so it lands at the same psum columns as
    {kw0,kw1}; the kw4 pair lands even-aligned at [0:64] (x cols 4:65 via a
    zero-padded 65th column) and odd-aligned at [64:128] (x cols 3:64).
    Combine is then 2 ops: ScalarE bias-add of the even half [.., 0:60] plus
    VectorE add of the odd half [.., 1:61].
  - Per group, all 8 row-chunks' M=128 matmuls are issued first (phase 1),
    then the M=64 kw4 pairs (phase 2), with one psum bank per row-chunk: the
    PE pays its ~110ns tile-config switch penalty twice per group instead of
    twice per row-chunk.
  - Head: critical DMAs split across BOTH HWDGE dispatch queues (SyncE for
    x bands, ScalarE/ACT for weights) so the serial ~590ns dispatch chains
    run in parallel; warmup matmul count sized so the PE queue drains right
    as the group-0 data lands (~11us), instead of over-warming.
  - Tail: last group's output DMAs alternate Sync/Scalar dispatch queues and
    the final row-chunk's odd-half add runs on the idle GpSimd engine so the
    drain after the last matmul is short.
  - PE warmed up with dummy matmuls during the initial DMA fill (HAM clock
    gate holds the array at 1.2 GHz until ~3.4us of sustained activity).
  - Compute in bf16 (host casts x/W), fp32 PSUM accumulate; rel err ~2e-3.

  Falsified alternatives (measured on HW, all reverted — do not retry):
  - Per-kw-block weight tiles (wa0/wa1/..): +120us — splitting the combined
    weight tiles breaks LDWEIGHTS/FWL overlap (+42ns on EVERY matmul).
  - Host-prewindowed pitch-61 x variants for flat (crossing-free) rhs:
    +152us — the 2.8x gather traffic stalls matmuls; prefetch cannot hide it.
  - fp8 (any hi/lo split passing rel-err 2e-2 is >=1.5x bf16 MACs), Winograd
    (transforms cannot ride the PE; DVE is 100x too slow), kh-in-K packing
    (needs 5x row-shifted x copies): all slower at equal accuracy.
"""

import os
import sys

import numpy as np
import ml_dtypes

for _p in ("/opt/trn_rl_repo", "/root/.axon_site/_ro/trn_rl_repo"):
    if os.path.isdir(_p) and _p not in sys.path:
        sys.path.append(_p)

import concourse.bass as bass
import concourse.mybir as mybir
import concourse.tile as tile
from concourse import bacc
from concourse.bass_utils import run_bass_kernel_spmd

BF16 = ml_dtypes.bfloat16

B, C, G_IN = 8, 32, 33
O, G_F, KH, KW = 64, 7, 5, 5
X, Y = 64, 64
G_OUT = 15
XO, YO = X - KH + 1, Y - KW + 1  # 60, 60
RCH = 8  # output rows per chunk (8*61 = 488 <= 512 psum bank)
N_WARM = 6  # dummy matmuls bridging program start to group-0 data-ready


def _build_nc(n_groups=G_OUT):
    """Build the single-core Bass program (x pre-gathered host-side)."""
    nc = bacc.Bacc("TRN2", target_bir_lowering=False, debug=False)
    dt = mybir.dt
    ga_d = nc.dram_tensor("ga", [G_OUT, 128, X, Y + 1], dt.bfloat16, kind="ExternalInput")
    gb_d = nc.dram_tensor("gb", [G_OUT, 96, X, Y + 1], dt.bfloat16, kind="ExternalInput")
    wa_d = nc.dram_tensor("wa", [128, KH, 5 * O], dt.bfloat16, kind="ExternalInput")
    wb_d = nc.dram_tensor("wb", [96, KH, 5 * O], dt.bfloat16, kind="ExternalInput")
    bias_d = nc.dram_tensor("bias", [O, 1], dt.float32, kind="ExternalInput")
    out_d = nc.dram_tensor("out", [O, G_OUT, XO, YO], dt.float32, kind="ExternalOutput")

    rchunks = [(r0, min(RCH, XO - r0)) for r0 in range(0, XO, RCH)]

    with tile.TileContext(nc) as tc:
        with (
            tc.tile_pool(name="wpool", bufs=1) as wpool,
            tc.tile_pool(name="warm", bufs=1) as warmpool,
            tc.tile_pool(name="xpool", bufs=2) as xpool,
            tc.tile_pool(name="tpool", bufs=3) as tpool,
            tc.tile_pool(name="opool", bufs=4) as opool,
            tc.tile_pool(name="psum", bufs=7, space="PSUM") as pp,
            tc.tile_pool(name="psumt", bufs=1, space="PSUM") as ppt,
        ):
            # PE warmup: the HAM clock gate holds the PE at 1.2 GHz until it
            # sees ~3.4us of sustained activity. Burn that in on garbage data
            # while the weight/x DMAs fill SBUF.
            wmt = warmpool.tile([128, 256], dt.bfloat16, tag="warm")
            nc.vector.memset(wmt[:, :], 0.0)

            # Weight DMAs ride the ScalarE HWDGE queue so their dispatch
            # chain runs in parallel with the x-band dispatches on SyncE,
            # split by kw-pair block in consumption order: the first real
            # matmuls need only wa01/wb cols 0:128 (280KB), not all 1.4MB.
            wa01 = wpool.tile([128, KH, 4 * O], dt.bfloat16, tag="wa01")
            wa4 = wpool.tile([128, KH, O], dt.bfloat16, tag="wa4")
            wb = wpool.tile([96, KH, 5 * O], dt.bfloat16, tag="wb")
            bias_sb = wpool.tile([O, 1], dt.float32, tag="bias")
            nc.scalar.dma_start(wa01[:, :, 0:128], wa_d[:, :, 0:128])
            nc.scalar.dma_start(wb[:, :, 0:128], wb_d[:, :, 0:128])
            nc.scalar.dma_start(wa01[:, :, 128:256], wa_d[:, :, 128:256])
            nc.scalar.dma_start(wb[:, :, 128:256], wb_d[:, :, 128:256])
            nc.scalar.dma_start(wa4[:, :, :], wa_d[:, :, 4 * O : 5 * O])
            nc.scalar.dma_start(wb[:, :, 256:320], wb_d[:, :, 256:320])
            nc.scalar.dma_start(bias_sb[:, :], bias_d[:, :])

            for i in range(n_groups):
                # xa has a 65th zeroed column so the kw4 matmul can stream a
                # full 61-wide window (cols 4:65) for a contiguous psum write.
                xa = xpool.tile([128, X, Y + 1], dt.bfloat16, tag="xa")
                xb = xpool.tile([96, X, Y + 1], dt.bfloat16, tag="xb")
                # group 0 in three row bands: the first (smallest) band
                # unblocks the first row chunk's matmuls as early as possible
                bands = ((0, 16), (16, 40), (40, X)) if i == 0 else ((0, X),)
                for lo, hi in bands:
                    nc.sync.dma_start(xa[:, lo:hi, :], ga_d[i, :, lo:hi, :])
                    nc.sync.dma_start(xb[:, lo:hi, :], gb_d[i, :, lo:hi, :])
                if i == 0:
                    # warmup matmuls run while the DMAs above land
                    wps = ppt.tile([128, 4, 61], dt.float32, tag="pt")
                    for _ in range(N_WARM):
                        nc.tensor.matmul(
                            wps[:, :, :],
                            wmt[:, 0:128],
                            wmt[:, 0:244],
                            start=True,
                            stop=True,
                        )

                # Phase 1: the M=128 {kw0,kw1}/{kw2,kw3} blocks of ALL row
                # chunks, one psum bank per chunk. Phase 2: the M=64 kw4
                # col-tiled pairs. Keeping all M=128 matmuls together avoids
                # the ~110ns PE tile-config switch penalty on every M change
                # (2 per group instead of 2 per row chunk).
                ptiles = []
                for r0, R in rchunks:
                    tail = R != RCH
                    p = (ppt if tail else pp).tile(
                        [128, R, 61], dt.float32, tag="pt" if tail else "p"
                    )
                    ptiles.append(p)
                    # {kw2,kw3} streams x cols 2:63 so its contributions land
                    # at the same psum columns as {kw0,kw1}.
                    for grp in (0, 1):
                        c0 = 2 * grp
                        for ci, (xt, wt, Kc) in enumerate(
                            ((xa, wa01, 128), (xb, wb, 96))
                        ):
                            for kh in range(KH):
                                nc.tensor.matmul(
                                    p[:, 0:R, :],
                                    wt[0:Kc, kh, grp * 128 : grp * 128 + 128],
                                    xt[0:Kc, r0 + kh : r0 + kh + R, c0 : c0 + 61],
                                    start=(grp == 0 and ci == 0 and kh == 0),
                                    stop=False,
                                )
                for ri, ((r0, R), p) in enumerate(zip(rchunks, ptiles)):
                    # kw4 col-tiled pair: even-aligned at [0:64] (x cols 4:65,
                    # zero-padded 65th col), odd-aligned at [64:128] (cols
                    # 3:64).
                    for kh in range(KH):
                        nc.tensor.matmul(
                            p[0:64, 0:R, :],
                            wa4[0:128, kh, 0:64],
                            xa[0:128, r0 + kh : r0 + kh + R, 4:65],
                            start=False,
                            stop=False,
                        )
                        nc.tensor.matmul(
                            p[64:128, 0:R, :],
                            wb[0:96, kh, 256:320],
                            xb[0:96, r0 + kh : r0 + kh + R, 3:64],
                            start=False,
                            stop=(kh == KH - 1),
                        )

                    # Combine (one PSUM operand per instruction): bias-add of
                    # the even half on ScalarE, odd half on VectorE.
                    # (GpSimd cannot read PSUM on TRN2, so no third engine.)
                    t = tpool.tile([O, RCH, 60], dt.float32, tag="t")
                    ot = opool.tile([O, RCH, 60], dt.float32, tag="out")
                    nc.scalar.add(t[:, 0:R, :], p[0:64, 0:R, 0:60], bias_sb[:, 0:1])
                    nc.vector.tensor_add(
                        ot[:, 0:R, :], t[:, 0:R, :], p[64:128, 0:R, 1:61]
                    )
                    # Output DMAs stay on SyncE: a scalar-queue dispatch's
                    # sem-wait blocks later ACT combines (measured +1.3us in
                    # the tail). Only the very last transfer splits across
                    # both queues so its HBM write-completion latency halves.
                    if i == n_groups - 1 and ri == len(rchunks) - 1:
                        rh = max(R // 2, 1)
                        nc.sync.dma_start(
                            out_d[:, i, r0 : r0 + rh, :], ot[:, 0:rh, :]
                        )
                        nc.scalar.dma_start(
                            out_d[:, i, r0 + rh : r0 + R, :], ot[:, rh:R, :]
                        )
                    else:
                        nc.sync.dma_start(
                            out_d[:, i, r0 : r0 + R, :], ot[:, 0:R, :]
                        )
    nc.compile()
    return nc


def _prep_inputs(x, weight, bias, idx):
    """Host-side staging: bf16 cast, idx gather, lhsT weight layout."""
    x16 = np.asarray(x).astype(BF16)  # [B, C, G_IN, X, Y]
    # pad a zero 65th column so kw4's 61-wide window (cols 4:65) exists
    x16 = np.pad(x16, ((0, 0), (0, 0), (0, 0), (0, 0), (0, 1)))
    w = np.asarray(weight).astype(np.float32)
    # lhsT layout: partition p = f*32 + c (within chunk), free = [kh, kw*64+o]
    wt = w.transpose(2, 1, 3, 4, 0)  # [G_F, C, KH, KW, O]
    wa = np.ascontiguousarray(wt[0:4].reshape(128, KH, KW * O)).astype(BF16)
    wb = np.ascontiguousarray(wt[4:7].reshape(96, KH, KW * O)).astype(BF16)
    b2 = np.ascontiguousarray(np.asarray(bias).astype(np.float32).reshape(O, 1))
    in_maps = []
    for b in range(B):
        # gather: g[i, f*32+c] = x16[b, c, idx[i, f]]
        gx = x16[b][:, idx]  # [C, G_OUT, G_F, X, Y+1]
        gx = gx.transpose(1, 2, 0, 3, 4).reshape(G_OUT, G_F * C, X, Y + 1)
        in_maps.append(
            {
                "ga": np.ascontiguousarray(gx[:, 0:128]),
                "gb": np.ascontiguousarray(gx[:, 128:224]),
                "wa": wa,
                "wb": wb,
                "bias": b2,
            }
        )
    return in_maps


def run(x, weight, bias, idx, trace=False):
    idx = np.asarray(idx).astype(np.int64)
    assert idx.shape == (G_OUT, G_F) and idx.min() >= 0 and idx.max() < G_IN
    nc = _build_nc()
    in_maps = _prep_inputs(x, weight, bias, idx)
    res = run_bass_kernel_spmd(nc, in_maps, list(range(B)), trace=trace)
    out = np.stack([res.results[b]["out"] for b in range(B)]).astype(np.float32)
    return out, res


def kernel(x, weight, bias, idx):
    out, _ = run(x, weight, bias, idx, trace=False)
    return out


# revision 13
# speedup vs baseline: 1.0645x; 1.0573x over previous
"""Trainium2 Bass kernel for nn_GroupLocalSL2 (grouped gather + conv).

out[b,o,i,xo,yo] = sum_{c,f,kh,kw} x[b,c,idx[i,f],xo+kh,yo+kw] * W[o,c,f,kh,kw] + bias[o]

Strategy:
  - Batch B=8 sharded across 8 NeuronCores (data parallel), one b per core.
  - Host pre-gathers x per output group (idx applied host-side) into
    contiguous [G_OUT, 128|96, X, Y+1] tensors so each group needs just two
    large DMAs instead of seven small gather DMAs (dma_start dispatch is
    ~590ns serial per issuing queue).
  - Per core: contraction (c,f)=224 split into chunkA (f=0..3, K=128) and
    chunkB (f=4..6, K=96), partition p = f*32 + c.
  - kw offsets packed into matmul M-blocks: {kw0,kw1} and {kw2,kw3} give
    M=128 matmuls; kw4 runs as two concurrent M=64 col-tiled matmuls. kh
    accumulates in PSUM via row-shifted rhs windows.
  - All 30 matmuls of a row-chunk accumulate into ONE psum tile [128, R, 61]:
    {kw2,kw3} streams x cols 2:63 so it lands at the same psum columns as
    {kw0,kw1}; the kw4 pair lands even-aligned at [0:64] (x cols 4:65 via a
    zero-padded 65th column) and odd-aligned at [64:128] (x cols 3:64).
    Combine is then 2 ops: ScalarE bias-add of the even half [.., 0:60] plus
    VectorE add of the odd half [.., 1:61].
  - Per group, all 8 row-chunks' M=128 matmuls are issued first (phase 1),
    then the M=64 kw4 pairs (phase 2), with one psum bank per row-chunk: the
    PE pays its ~110ns tile-config switch penalty twice per group instead of
    twice per row-chunk.
  - Head: critical DMAs split across BOTH HWDGE dispatch queues (SyncE for
    x bands, ScalarE/ACT for weights) so the serial ~590ns dispatch chains
    run in parallel; warmup matmul count sized so the PE queue drains right
    as the group-0 data lands (~11us), instead of over-warming.
  - Tail: last group's output DMAs alternate Sync/Scalar dispatch queues and
    the final row-chunk's odd-half add runs on the idle GpSimd engine so the
    drain after the last matmul is short.
  - PE warmed up with dummy matmuls during the initial DMA fill (HAM clock
    gate holds the array at 1.2 GHz until ~3.4us of sustained activity).
  - Compute in bf16 (host casts x/W), fp32 PSUM accumulate; rel err ~2e-3.

  Falsified alternatives (measured on HW, all reverted — do not retry):
  - Per-kw-block weight tiles (wa0/wa1/..): +120us — splitting the combined
    weight tiles breaks LDWEIGHTS/FWL overlap (+42ns on EVERY matmul).
  - Host-prewindowed pitch-61 x variants for flat (crossing-free) rhs:
    +152us — the 2.8x gather traffic stalls matmuls; prefetch cannot hide it.
  - fp8 (any hi/lo split passing rel-err 2e-2 is >=1.5x bf16 MACs), Winograd
    (transforms cannot ride the PE; DVE is 100x too slow), kh-in-K packing
    (needs 5x row-shifted x copies): all slower at equal accuracy.
"""

import os
import sys

import numpy as np
import ml_dtypes

for _p in ("/opt/trn_rl_repo", "/root/.axon_site/_ro/trn_rl_repo"):
    if os.path.isdir(_p) and _p not in sys.path:
        sys.path.append(_p)

import concourse.bass as bass
import concourse.mybir as mybir
import concourse.tile as tile
from concourse import bacc
from concourse.bass_utils import run_bass_kernel_spmd

BF16 = ml_dtypes.bfloat16

B, C, G_IN = 8, 32, 33
O, G_F, KH, KW = 64, 7, 5, 5
X, Y = 64, 64
G_OUT = 15
XO, YO = X - KH + 1, Y - KW + 1  # 60, 60
RCH = 8  # output rows per chunk (8*61 = 488 <= 512 psum bank)
N_WARM = 18  # dummy matmuls bridging program start to group-0 data-ready


def _build_nc(n_groups=G_OUT):
    """Build the single-core Bass program (x pre-gathered host-side)."""
    nc = bacc.Bacc("TRN2", target_bir_lowering=False, debug=False)
    dt = mybir.dt
    ga_d = nc.dram_tensor("ga", [G_OUT, 128, X, Y + 1], dt.bfloat16, kind="ExternalInput")
    gb_d = nc.dram_tensor("gb", [G_OUT, 96, X, Y + 1], dt.bfloat16, kind="ExternalInput")
    # all weights packed in one tensor: rows 0:5 = wa[kh], rows 5:10 = wb[kh]
    # (partitions 96:128 of the wb rows are zero-padded)
    wt_d = nc.dram_tensor("wt", [128, 2 * KH, KW * O], dt.bfloat16, kind="ExternalInput")
    bias_d = nc.dram_tensor("bias", [O, 1], dt.float32, kind="ExternalInput")
    out_d = nc.dram_tensor("out", [O, G_OUT, XO, YO], dt.float32, kind="ExternalOutput")

    rchunks = [(r0, min(RCH, XO - r0)) for r0 in range(0, XO, RCH)]

    with tile.TileContext(nc) as tc:
        with (
            tc.tile_pool(name="wpool", bufs=1) as wpool,
            tc.tile_pool(name="warm", bufs=1) as warmpool,
            tc.tile_pool(name="xpool", bufs=2) as xpool,
            tc.tile_pool(name="tpool", bufs=3) as tpool,
            tc.tile_pool(name="opool", bufs=4) as opool,
            tc.tile_pool(name="psum", bufs=7, space="PSUM") as pp,
            tc.tile_pool(name="psumt", bufs=1, space="PSUM") as ppt,
        ):
            # PE warmup: the HAM clock gate holds the PE at 1.2 GHz until it
            # sees ~3.4us of sustained activity. Burn that in on garbage data
            # while the weight/x DMAs fill SBUF.
            wmt = warmpool.tile([128, 256], dt.bfloat16, tag="warm")
            nc.vector.memset(wmt[:, :], 0.0)

            # Weight DMAs ride the ScalarE HWDGE queue so they can't be
            # starved by the group-1 x prefetch on SyncE (measured: fine-
            # grained weight splits arrive LATE because each HWDGE transfer
            # pays ~1.3us fixed latency and shares HBM with the prefetch).
            # Two DMAs: kw-pair-0 block first (needed by matmul #1), rest
            # second.
            wt = wpool.tile([128, 2 * KH, KW * O], dt.bfloat16, tag="wt")
            bias_sb = wpool.tile([O, 1], dt.float32, tag="bias")
            nc.scalar.dma_start(wt[:, :, 0:128], wt_d[:, :, 0:128])
            nc.scalar.dma_start(wt[:, :, 128:320], wt_d[:, :, 128:320])
            nc.scalar.dma_start(bias_sb[:, :], bias_d[:, :])

            for i in range(n_groups):
                # xa has a 65th zeroed column so the kw4 matmul can stream a
                # full 61-wide window (cols 4:65) for a contiguous psum write.
                xa = xpool.tile([128, X, Y + 1], dt.bfloat16, tag="xa")
                xb = xpool.tile([96, X, Y + 1], dt.bfloat16, tag="xb")
                # group 0 in three row bands: the first (smallest) band
                # unblocks the first row chunk's matmuls as early as possible
                bands = ((0, 16), (16, 40), (40, X)) if i == 0 else ((0, X),)
                for lo, hi in bands:
                    nc.sync.dma_start(xa[:, lo:hi, :], ga_d[i, :, lo:hi, :])
                    nc.sync.dma_start(xb[:, lo:hi, :], gb_d[i, :, lo:hi, :])
                if i == 0:
                    # warmup matmuls run while the DMAs above land
                    wps = ppt.tile([128, 4, 61], dt.float32, tag="pt")
                    for _ in range(N_WARM):
                        nc.tensor.matmul(
                            wps[:, :, :],
                            wmt[:, 0:128],
                            wmt[:, 0:244],
                            start=True,
                            stop=True,
                        )

                # Phase 1: the M=128 {kw0,kw1}/{kw2,kw3} blocks of ALL row
                # chunks, one psum bank per chunk. Phase 2: the M=64 kw4
                # col-tiled pairs. Keeping all M=128 matmuls together avoids
                # the ~110ns PE tile-config switch penalty on every M change
                # (2 per group instead of 2 per row chunk).
                ptiles = []
                for r0, R in rchunks:
                    tail = R != RCH
                    p = (ppt if tail else pp).tile(
                        [128, R, 61], dt.float32, tag="pt" if tail else "p"
                    )
                    ptiles.append(p)
                    # {kw2,kw3} streams x cols 2:63 so its contributions land
                    # at the same psum columns as {kw0,kw1}.
                    for grp in (0, 1):
                        c0 = 2 * grp
                        for ci, (xt, kh0, Kc) in enumerate(
                            ((xa, 0, 128), (xb, KH, 96))
                        ):
                            for kh in range(KH):
                                nc.tensor.matmul(
                                    p[:, 0:R, :],
                                    wt[0:Kc, kh0 + kh, grp * 128 : grp * 128 + 128],
                                    xt[0:Kc, r0 + kh : r0 + kh + R, c0 : c0 + 61],
                                    start=(grp == 0 and ci == 0 and kh == 0),
                                    stop=False,
                                )
                for ri, ((r0, R), p) in enumerate(zip(rchunks, ptiles)):
                    # kw4 col-tiled pair: even-aligned at [0:64] (x cols 4:65,
                    # zero-padded 65th col), odd-aligned at [64:128] (cols
                    # 3:64).
                    for kh in range(KH):
                        nc.tensor.matmul(
                            p[0:64, 0:R, :],
                            wt[0:128, kh, 256:320],
                            xa[0:128, r0 + kh : r0 + kh + R, 4:65],
                            start=False,
                            stop=False,
                        )
                        nc.tensor.matmul(
                            p[64:128, 0:R, :],
                            wt[0:96, KH + kh, 256:320],
                            xb[0:96, r0 + kh : r0 + kh + R, 3:64],
                            start=False,
                            stop=(kh == KH - 1),
                        )

                    # Combine (one PSUM operand per instruction): bias-add of
                    # the even half on ScalarE, odd half on VectorE.
                    # (GpSimd cannot read PSUM on TRN2, so no third engine.)
                    t = tpool.tile([O, RCH, 60], dt.float32, tag="t")
                    ot = opool.tile([O, RCH, 60], dt.float32, tag="out")
                    nc.scalar.add(t[:, 0:R, :], p[0:64, 0:R, 0:60], bias_sb[:, 0:1])
                    nc.vector.tensor_add(
                        ot[:, 0:R, :], t[:, 0:R, :], p[64:128, 0:R, 1:61]
                    )
                    # Output DMAs stay on SyncE: a scalar-queue dispatch's
                    # sem-wait blocks later ACT combines (measured +1.3us in
                    # the tail). Only the very last transfer splits across
                    # both queues so its HBM write-completion latency halves.
                    if i == n_groups - 1 and ri == len(rchunks) - 1:
                        rh = max(R // 2, 1)
                        nc.sync.dma_start(
                            out_d[:, i, r0 : r0 + rh, :], ot[:, 0:rh, :]
                        )
                        nc.scalar.dma_start(
                            out_d[:, i, r0 + rh : r0 + R, :], ot[:, rh:R, :]
                        )
                    else:
                        nc.sync.dma_start(
                            out_d[:, i, r0 : r0 + R, :], ot[:, 0:R, :]
                        )
    nc.compile()
    return nc


def _prep_inputs(x, weight, bias, idx):
    """Host-side staging: bf16 cast, idx gather, lhsT weight layout."""
    x16 = np.asarray(x).astype(BF16)  # [B, C, G_IN, X, Y]
    # pad a zero 65th column so kw4's 61-wide window (cols 4:65) exists
    x16 = np.pad(x16, ((0, 0), (0, 0), (0, 0), (0, 0), (0, 1)))
    w = np.asarray(weight).astype(np.float32)
    # lhsT layout: partition p = f*32 + c (within chunk), free = [kh, kw*64+o]
    wx = w.transpose(2, 1, 3, 4, 0)  # [G_F, C, KH, KW, O]
    wa = wx[0:4].reshape(128, KH, KW * O)
    wb = wx[4:7].reshape(96, KH, KW * O)
    # packed weight tensor: rows 0:5 = wa kh-slices, rows 5:10 = wb kh-slices
    # (partitions 96:128 zero there)
    wfull = np.zeros((128, 2 * KH, KW * O), dtype=np.float32)
    wfull[:, 0:KH] = wa
    wfull[0:96, KH : 2 * KH] = wb
    wfull = np.ascontiguousarray(wfull).astype(BF16)
    b2 = np.ascontiguousarray(np.asarray(bias).astype(np.float32).reshape(O, 1))
    in_maps = []
    for b in range(B):
        # gather: g[i, f*32+c] = x16[b, c, idx[i, f]]
        gx = x16[b][:, idx]  # [C, G_OUT, G_F, X, Y+1]
        gx = gx.transpose(1, 2, 0, 3, 4).reshape(G_OUT, G_F * C, X, Y + 1)
        in_maps.append(
            {
                "ga": np.ascontiguousarray(gx[:, 0:128]),
                "gb": np.ascontiguousarray(gx[:, 128:224]),
                "wt": wfull,
                "bias": b2,
            }
        )
    return in_maps


def run(x, weight, bias, idx, trace=False):
    idx = np.asarray(idx).astype(np.int64)
    assert idx.shape == (G_OUT, G_F) and idx.min() >= 0 and idx.max() < G_IN
    nc = _build_nc()
    in_maps = _prep_inputs(x, weight, bias, idx)
    res = run_bass_kernel_spmd(nc, in_maps, list(range(B)), trace=trace)
    out = np.stack([res.results[b]["out"] for b in range(B)]).astype(np.float32)
    return out, res


def kernel(x, weight, bias, idx):
    out, _ = run(x, weight, bias, idx, trace=False)
    return out
